# revision 53
# baseline (speedup 1.0000x reference)
"""Trainium2 Bass kernel for LowLightAdaptiveCNNBlock (moe_routing).

Full inputs in, full outputs out. Data-parallel over batch on 8 NeuronCores
(2 samples/core), params replicated.

Per-core program (SPMD), v2 — 6-pass tap-paired conv:
  - x shipped as fp8e4m3 pair (x8, r8 = fp8(x - x8)). SBUF planes are
    row-padded: plane = 98 rows x 98 cols, image row r at (r+1)*98, rows 0
    and 97 memset to zero, cols 0/97 zero from host. All conv chunks are
    then uniform -- taps read zeros at the borders exactly like the
    reference's zero padding (no row clipping anywhere).
  - main depthwise 3x3 conv: 6 DoubleRow matmul passes per chunk instead of
    10. Each DR pass contracts TWO tap-shifted views of the x8 plane in one
    matmul by giving the rhs pair dimension an explicit element stride
    (pair stride >= 2 required by HW; stride 1 wedges the device):
      P_res (1,1) x (x8(0,0), r8(0,0))          pair stride PL2
      P0 (k00,k02) x (x8(-1,-1), x8(-1,+1))     pair stride 2
      P1 (k10,k12) x (x8( 0,-1), x8( 0,+1))     pair stride 2
      P2 (k20,k22) x (x8(+1,-1), x8(+1,+1))     pair stride 2
      P3 (k01,k11) x (x8(-1, 0), x8( 0, 0))     pair stride 98
      P4 (k21,k11r) x (x8(+1,0), r8( 0, 0))     pair stride PL2-98
    k11r = center tap applied to r8 (the largest fp8-residual correction);
    the other 8 taps drop their r8 term (small vs the fp8 weight noise).
    BN scale folded into tap weights; epilogue relu(psum + off) -> bf16.
  - routing path via pooled-sum algebra on the x8 plane: mean(BN(dwconv+b))
    is linear in 16 per-channel sums (8 eighth-plane sums, border rows/
    cols, corners). Sample-0 x8 arrives as 16 eighth-DMAs raced on the
    sync (cb0) and gpsimd (cb1) queues; eighth reduces chase arrivals on
    Vector/Scalar/GpSimd round-robin. Consts ride one packed blob DMA on
    the scalar queue.
  - softmax routing weights -> expert kernel/bias mixing via tiny bf16 PE
    matmuls; mixed tap weights written as fp8 diag half-planes from an
    fp8 [I|I] constant (one [128,128] write per tap slot).

Ordering note: cross-engine waits compile to per-engine position counters,
so consumers wait for everything issued earlier on the producer engine.
Sample 1's VectorE routing work is issued only after all of sample 0's
VectorE work and drains under sample 0's conv (ScalarE-only evacuation).
"""
import sys

sys.path.insert(0, "/opt/trn_rl_repo")

import numpy as np
import concourse.bass as bass
import concourse.bacc as bacc
import concourse.tile as tile
from concourse import mybir
from concourse.ap import AP
from concourse.bass_utils import run_bass_kernel_spmd

f32 = mybir.dt.float32
f8 = mybir.dt.float8e4
bf16 = mybir.dt.bfloat16
u8 = mybir.dt.uint8

EPS = 1e-5
B, C, H, W, E = 16, 256, 96, 96, 8
NCORES = 8
BLOC = B // NCORES          # samples per core
NCB = C // 128              # channel blocks
WP = W + 2                  # padded row stride (98)
HP = H + 2                  # padded rows (98)
PL2 = WP * HP               # 9604 = padded plane span
XTD = WP * H                # 9408 = data span (rows 1..96 of plane)
XE = XTD // 8               # 1176 = eighth piece (12 image rows)
ROWCH = 5                   # rows per conv chunk
GROUP = 3                   # chunks per PSUM group
HW = H * W

XQ = XTD // 4               # 2352 = quarter piece (24 image rows)

# const blob byte offsets (all 4-aligned)
OFF_PP = (0, 56)            # per cb: [128, 14] f32
OFF_PPB = (112, 128)        # per cb: [128, 8] bf16
OFF_DB = 144                # [1, 8] f32 (row 0)
OFF_ID8 = 176               # [128, 256] f8
OFF_EMB = 432               # [8, NCB*10*128] bf16 (rows 0..7)
BLOB_B = OFF_EMB + NCB * 10 * 128 * 2   # 5552

CHUNKS = []
_r = 0
while _r < H:
    _nr = min(ROWCH, H - _r)
    CHUNKS.append((_r, _nr))
    _r += _nr

# conv pass table: (lhs half0 tap, lhs half1 tap, rhs base (sh,sw), pair Δ)
# tap index t = (sh+1)*3 + (sw+1); -1 = x8 residual (weight 1). All lhs
# weights are scaled x16 (id8 = 16[I|I]) so small mixed taps stay out of
# fp8 subnormals; the epilogue rescales by 1/16. The r8 residual plus the
# center-tap r8 correction are applied in the epilogue as (16+16*k4)*r8.
PASSES = [
    (0, 2, (-1, -1), 2),
    (3, 5, (0, -1), 2),
    (6, 8, (1, -1), 2),
    (1, 4, (-1, 0), WP),
    (-1, 7, (0, 0), WP),    # (residual x8, k7 @ (+1,0))
]


def _build_program():
    nc = bacc.Bacc("TRN2", target_bir_lowering=False, debug=False,
                   num_devices=NCORES)

    x8_d = nc.dram_tensor("x8", [BLOC, C, H, WP], f8,
                          kind="ExternalInput").ap()
    r8_d = nc.dram_tensor("r8", [BLOC, C, H, WP], f8,
                          kind="ExternalInput").ap()
    blob_d = nc.dram_tensor("blob", [128, BLOB_B], u8,
                            kind="ExternalInput").ap()
    out_d = nc.dram_tensor("out", [BLOC, C, H, W], bf16,
                           kind="ExternalOutput").ap()

    Relu = mybir.ActivationFunctionType.Relu
    Exp = mybir.ActivationFunctionType.Exp
    Copy = mybir.ActivationFunctionType.Copy
    add_op = mybir.AluOpType.add
    mult_op = mybir.AluOpType.mult
    max_op = mybir.AluOpType.max
    AX = mybir.AxisListType.X
    AXY = mybir.AxisListType.XY
    DR = mybir.MatmulPerfMode.DoubleRow

    with tile.TileContext(nc) as tc:
        with tc.tile_pool(name="const", bufs=1) as cpool, \
             tc.tile_pool(name="xp", bufs=4) as xpool, \
             tc.tile_pool(name="small", bufs=4) as spool, \
             tc.tile_pool(name="rp", bufs=1) as rpool, \
             tc.tile_pool(name="dg", bufs=4) as dgpool, \
             tc.tile_pool(name="st", bufs=8) as stpool, \
             tc.tile_pool(name="tm", bufs=8) as tmpool, \
             tc.tile_pool(name="s32", bufs=14) as s32pool, \
             tc.tile_pool(name="sc", bufs=1) as scpool, \
             tc.tile_pool(name="pc", bufs=7, space="PSUM") as pconv, \
             tc.tile_pool(name="pxs", bufs=1, space="PSUM") as psmall:

            xt = [[xpool.tile([128, 2 * PL2], f8, tag="xt",
                              name=f"xt{b}{cb}")
                   for cb in range(NCB)] for b in range(BLOC)]

            # zero the pad rows (rows 0 and 97 of both planes) of the
            # sample-0 tiles. Disjoint from the DMA'd interior, so these
            # overlap the transfers. Sample-1 pads are zeroed later (not
            # needed until its conv) to keep gpsimd's DMA issues early.
            for cb in range(NCB):
                t = xt[0][cb]
                for pl in (0, PL2):
                    nc.vector.memset(t[:, pl:pl + WP], 0.0)
                    nc.vector.memset(t[:, pl + 97 * WP:pl + 98 * WP], 0.0)

            blob_t = cpool.tile([128, BLOB_B], u8, tag="blob")
            pp_t = [blob_t[:, OFF_PP[cb]:OFF_PP[cb] + 56].bitcast(f32)
                    for cb in range(NCB)]
            ppb_t = [blob_t[:, OFF_PPB[cb]:OFF_PPB[cb] + 16].bitcast(bf16)
                     for cb in range(NCB)]
            db_t = blob_t[0:1, OFF_DB:OFF_DB + 32].bitcast(f32)
            id8_t = blob_t[:, OFF_ID8:OFF_ID8 + 256].bitcast(f8)
            emb_t = blob_t[0:8, OFF_EMB:BLOB_B].bitcast(bf16)

            # ---- x8 sample-0 quarter pieces raced on the sync (cb0) and
            # scalar (cb1, behind the blob) HWDGE queues; r8 s0 + all of
            # s1 trail on the gpsimd software queue.
            # The HWDGE queues (sync, scalar) have ~1.5us issue-to-data
            # latency; the gpsimd software queue has ~11us latency but
            # high throughput. Startup-critical x8 sample-0 halves ride
            # the HW queues (3 DMAs each -- more stalls the sem ring);
            # everything else rides gpsimd and lands from ~19us on.
            # x8 s0 halves on the low-latency HW queues (h0 consumed from
            # ~14us, h1 from ~19); everything else on the gpsimd software
            # queue (~11us latency), ordered by need time
            sl_h0 = slice(WP, WP + 2 * XQ)
            sl_h1 = slice(WP + 2 * XQ, WP + 4 * XQ)
            nc.sync.dma_start(xt[0][0][:, sl_h0], x8_d[0, 0:128, 0:48])
            nc.scalar.dma_start(xt[0][1][:, sl_h0], x8_d[0, 128:256, 0:48])
            nc.sync.dma_start(xt[0][0][:, sl_h1], x8_d[0, 0:128, 48:96])
            nc.scalar.dma_start(xt[0][1][:, sl_h1], x8_d[0, 128:256, 48:96])
            nc.gpsimd.dma_start(blob_t[:], blob_d)
            for cb in range(NCB):
                s = slice(cb * 128, (cb + 1) * 128)
                nc.gpsimd.dma_start(xt[0][cb][:, PL2 + WP:PL2 + WP + XTD],
                                    r8_d[0, s])
            for cb in range(NCB):
                s = slice(cb * 128, (cb + 1) * 128)
                nc.gpsimd.dma_start(xt[1][cb][:, WP:WP + XTD], x8_d[1, s])
            for cb in range(NCB):
                s = slice(cb * 128, (cb + 1) * 128)
                nc.gpsimd.dma_start(xt[1][cb][:, PL2 + WP:PL2 + WP + XTD],
                                    r8_d[1, s])

            ones_t = cpool.tile([1, 1], bf16, tag="ones")
            nc.vector.memset(ones_t[:], 1.0)
            sink_s = scpool.tile([128, XQ], f8, tag="sinks")

            r_t = [[rpool.tile([128, 1], bf16, tag=f"r{b}{cb}",
                               name=f"r_t{b}{cb}")
                    for cb in range(NCB)] for b in range(BLOC)]
            U_t = [[None] * NCB for _ in range(BLOC)]

            def alloc_U(b, cb):
                U = spool.tile([128, 16], f32, tag=f"U{b}{cb}",
                               name=f"U{b}{cb}")
                U_t[b][cb] = U
                return U

            def reduce_quarter(b, cb, q, eng):
                t = xt[b][cb]
                U = U_t[b][cb]
                src = t[:, WP + q * XQ:WP + (q + 1) * XQ]
                if eng == "vector":
                    nc.vector.tensor_reduce(U[:, q:q + 1], src, axis=AX,
                                            op=add_op)
                else:
                    nc.scalar.activation(sink_s[:], src, Copy,
                                         accum_out=U[:, q:q + 1])

            def small_stats_v(b, cb):
                """border stats on VectorE.
                U layout: [Q0..Q3, R0, R95, C0, C95, x00, x0_95, x95_0,
                x95_95]"""
                t = xt[b][cb]
                U = U_t[b][cb]
                colv = t[:, WP:WP + XTD].rearrange("p (r c) -> p r c", c=WP)
                nc.vector.tensor_reduce(U[:, 4:6], colv[:, 0:96:95, 1:97],
                                        axis=AX, op=add_op)
                nc.vector.tensor_reduce(U[:, 6:7], colv[:, :, 1:2],
                                        axis=AXY, op=add_op)
                nc.vector.tensor_reduce(U[:, 7:8], colv[:, :, 96:97],
                                        axis=AXY, op=add_op)
                nc.vector.tensor_copy(U[:, 8:12], colv[:, 0:96:95, 1:97:95])

            def small_stats_s(b, cb):
                """border stats on ScalarE via activation accumulate."""
                t = xt[b][cb]
                U = U_t[b][cb]
                colv = t[:, WP:WP + XTD].rearrange("p (r c) -> p r c", c=WP)
                snk = sink_s[:, 0:96]
                nc.scalar.activation(snk, colv[:, 0:1, 1:97], Copy,
                                     accum_out=U[:, 4:5])
                nc.scalar.activation(snk, colv[:, 95:96, 1:97], Copy,
                                     accum_out=U[:, 5:6])
                nc.scalar.activation(snk, colv[:, :, 1:2], Copy,
                                     accum_out=U[:, 6:7])
                nc.scalar.activation(snk, colv[:, :, 96:97], Copy,
                                     accum_out=U[:, 7:8])
                nc.scalar.activation(U[:, 8:12], colv[:, 0:96:95, 1:97:95],
                                     Copy)

            def combine_r(b, cb):
                """U * pp -> pooled mean -> relu -> r_t (VectorE)."""
                U = U_t[b][cb]
                scr = spool.tile([128, 12], f32, tag="scr")
                m_t = spool.tile([128, 1], f32, tag="m")
                nc.vector.tensor_tensor(scr[:], U[:, 0:12],
                                        pp_t[cb][:, 0:12], op=mult_op)
                nc.vector.tensor_reduce(m_t[:], scr[:], axis=AX, op=add_op)
                nc.vector.tensor_scalar(r_t[b][cb][:], m_t[:],
                                        pp_t[cb][:, 12:13], 0.0,
                                        op0=add_op, op1=max_op)

            def routing_tail(b):
                pl = psmall.tile([1, E], f32, tag="psm")
                for cb in range(NCB):
                    nc.tensor.matmul(pl[:], r_t[b][cb][:], ppb_t[cb],
                                     start=(cb == 0), stop=(cb == NCB - 1))
                lg = spool.tile([1, E], f32, tag="lg")
                nc.vector.tensor_tensor(lg[:], pl[:], db_t, op=add_op)
                # logits are O(+-1): exp without max-subtraction is safe
                ex = spool.tile([1, E], f32, tag="ex")
                nc.scalar.activation(ex[:], lg[:], Exp, bias=0.0, scale=1.0)
                sm = spool.tile([1, 1], f32, tag="sm")
                nc.vector.reduce_sum(sm[:], ex[:], axis=AX)
                rs = spool.tile([1, 1], f32, tag="rs")
                nc.vector.reciprocal(rs[:], sm[:])
                wsm = spool.tile([1, E], bf16, tag="wsm")
                nc.vector.tensor_scalar_mul(wsm[:], ex[:], rs[:])
                pw = psmall.tile([E, 1], f32, tag="psm")
                nc.tensor.matmul(pw[:], wsm[:], ones_t[:], start=True,
                                 stop=True)
                wT = spool.tile([E, 1], bf16, tag="wT")
                nc.vector.tensor_copy(wT[:], pw[:])
                return wT

            def emb_sl(cb, t):
                return emb_t[:, (cb * 10 + t) * 128:(cb * 10 + t + 1) * 128]

            def mix_kb(b, cb, wT, diag_engine):
                """mixed+scaled tap weights -> fp8 diag half planes
                [128, 10*128] (5 DR pairs) and off [128, 1]."""
                pk = psmall.tile([128, 10], f32, tag="psm")
                for t in range(10):
                    nc.tensor.matmul(pk[:, t:t + 1], emb_sl(cb, t), wT[:],
                                     start=True, stop=True)
                kb = spool.tile([128, 10], f32, tag="kb")
                nc.vector.tensor_copy(kb[:], pk[:])
                off = spool.tile([128, 1], f32, tag="off")
                nc.vector.tensor_tensor(off[:], kb[:, 9:10],
                                        pp_t[cb][:, 13:14], op=add_op)
                diag = dgpool.tile([128, 10 * 128], f8, tag="diag")

                def eng_for(i):
                    if diag_engine == "vector":
                        return nc.vector
                    if diag_engine == "scalar":
                        return nc.scalar
                    if diag_engine == "tri":
                        return (nc.vector, nc.scalar, nc.gpsimd)[i % 3]
                    return nc.vector if i % 2 == 0 else nc.scalar

                half_taps = []
                for (ta, tb, _, _) in PASSES:
                    half_taps += [ta, tb]
                id1 = id8_t[:, 0:128]
                for i, t in enumerate(half_taps):
                    dst = diag[:, i * 128:(i + 1) * 128]
                    sc = 1.0 if t < 0 else kb[:, t:t + 1]
                    eng = eng_for(i)
                    if eng is nc.scalar:
                        nc.scalar.activation(dst, id1, Copy, bias=0.0,
                                             scale=sc)
                    else:
                        eng.tensor_scalar_mul(dst, id1, sc)
                # epilogue r8 coefficient: (1 + k4) * 16 per channel
                k4p = spool.tile([128, 1], f32, tag="k4p")
                nc.vector.tensor_scalar(k4p[:], kb[:, 4:5], 1.0, 16.0,
                                        op0=add_op, op1=mult_op)
                return diag, off, k4p

            def pair_rhs(t, base, delta, nr):
                return AP(t.tensor, t.offset + base,
                          [list(t.ap[0]), [delta, 2], [WP, nr], [1, 96]])

            def conv_tile(b, cb, diag, off, k4p):
                t = xt[b][cb]

                ng = (len(CHUNKS) + GROUP - 1) // GROUP
                for g in range(ng):
                    gchunks = CHUNKS[g * GROUP:(g + 1) * GROUP]
                    pss = []
                    for ci in range(len(gchunks)):
                        pss.append(pconv.tile([128, ROWCH * W], f32,
                                              tag="pc", name=f"ps{ci}"))
                    # r8 * (16 + 16*k4) on GpSimd, ahead of the matmul stops
                    tmps = []
                    for ci, (r0, nr) in enumerate(gchunks):
                        n = nr * W
                        tmp = tmpool.tile([128, ROWCH * W], f32, tag="tmp")
                        r8v = AP(t.tensor,
                                 t.offset + PL2 + WP * (r0 + 1) + 1,
                                 [list(t.ap[0]), [WP, nr], [1, 96]])
                        nc.gpsimd.tensor_scalar(tmp[:, 0:n], r8v,
                                                k4p[:, 0:1], 0.0,
                                                op0=mult_op, op1=add_op)
                        tmps.append(tmp)
                    for pi, (ta, tb, (sh, sw), delta) in enumerate(PASSES):
                        lhs = diag[:, 2 * pi * 128:
                                   (2 * pi + 2) * 128].rearrange(
                                       "p (two m) -> p two m", two=2)
                        for ci, (r0, nr) in enumerate(gchunks):
                            base = WP * (r0 + sh + 1) + 1 + sw
                            nc.tensor.matmul(
                                pss[ci][:, 0:nr * W], lhs,
                                pair_rhs(t, base, delta, nr),
                                start=(pi == 0),
                                stop=(pi == len(PASSES) - 1),
                                perf_mode=DR)
                    for ci, (r0, nr) in enumerate(gchunks):
                        n = nr * W
                        st32 = s32pool.tile([128, ROWCH * W], f32,
                                            tag="st32")
                        nc.vector.tensor_tensor(st32[:, 0:n],
                                                pss[ci][:, 0:n],
                                                tmps[ci][:, 0:n], op=add_op)
                        stage = stpool.tile([128, ROWCH * W], bf16, tag="st")
                        nc.scalar.activation(stage[:, 0:n], st32[:, 0:n],
                                             Relu, bias=off[:],
                                             scale=0.0625)
                        nc.sync.dma_start(
                            out_d[b, cb * 128:(cb + 1) * 128, r0:r0 + nr],
                            stage[:, 0:n])

            # ---- sample-0 routing: quarter reduces chase the DMA arrivals
            for cb in range(NCB):
                alloc_U(0, cb)
            ENG2 = ("vector", "scalar")
            for q in range(4):
                for cb in range(NCB):
                    reduce_quarter(0, cb, q, ENG2[(q + cb) % 2])
            small_stats_v(0, 0)
            small_stats_v(0, 1)
            combine_r(0, 0)
            combine_r(0, 1)

            wT0 = routing_tail(0)
            dg00 = mix_kb(0, 0, wT0, "tri")
            dg01 = mix_kb(0, 1, wT0, "tri")

            # sample-1 routing on VectorE only, held back so the static
            # scheduler doesn't weave it into sample 0's startup chain
            with tc.tile_wait_until(0.045):
                for cb in range(NCB):
                    alloc_U(1, cb)
                # sample-1 sums on ScalarE (VectorE is busy evacuating
                # PSUM for the 5-pass epilogue; the deep st32 pool absorbs
                # ScalarE's pause in the relu/stage stream)
                for q in range(4):
                    for cb in range(NCB):
                        reduce_quarter(1, cb, q, "scalar")
                small_stats_v(1, 0)
                small_stats_v(1, 1)
                combine_r(1, 0)
                combine_r(1, 1)

            # sample-1 pad rows (gpsimd, after its DMA issues)
            for cb in range(NCB):
                t = xt[1][cb]
                for pl in (0, PL2):
                    nc.gpsimd.memset(t[:, pl:pl + WP], 0.0)
                    nc.gpsimd.memset(t[:, pl + 97 * WP:pl + 98 * WP], 0.0)

            conv_tile(0, 0, *dg00)

            wT1 = routing_tail(1)
            dg10 = mix_kb(1, 0, wT1, "vector")
            dg11 = mix_kb(1, 1, wT1, "vector")

            conv_tile(0, 1, *dg01)
            conv_tile(1, 0, *dg10)
            conv_tile(1, 1, *dg11)

    if not nc.is_finalized():
        nc.finalize()
    return nc


_NC = None


def _get_nc():
    global _NC
    if _NC is None:
        _NC = _build_program()
    return _NC


def _host_prep(kernel_embed, bias_embed, cls_conv_w, cls_conv_b,
               cls_bn_gamma, cls_bn_beta, cls_bn_mean, cls_bn_var,
               cls_dense_w, cls_dense_b, bn_gamma, bn_beta, bn_mean, bn_var):
    from concourse.mybir import dt as _dt
    bf16np = _dt.np(bf16)
    f8np = _dt.np(f8)

    inv = bn_gamma / np.sqrt(bn_var + EPS)
    shift = bn_beta - bn_mean * inv
    cls_inv = cls_bn_gamma / np.sqrt(cls_bn_var + EPS)
    cls_shift = cls_bn_beta - cls_bn_mean * cls_inv

    cls_w9 = cls_conv_w.reshape(C, 9)
    A = (cls_inv[:, None] * cls_w9) / HW                      # (C, 9)
    d = cls_inv * cls_conv_b + cls_shift                      # (C,)

    # window-sum decomposition: S_ij = T - rho - gamma + kappa
    # stat order: [Q0..Q3, R0, R95, C0, C95, x00, x0_95, x95_0, x95_95]
    C12 = np.zeros((C, 12), np.float64)
    for i, sh in enumerate((-1, 0, 1)):
        for j, sw in enumerate((-1, 0, 1)):
            a = A[:, i * 3 + j].astype(np.float64)
            for q in range(4):
                C12[:, q] += a
            if sh == -1:
                C12[:, 5] -= a
            elif sh == 1:
                C12[:, 4] -= a
            if sw == -1:
                C12[:, 7] -= a
            elif sw == 1:
                C12[:, 6] -= a
            if sh != 0 and sw != 0:
                rr = 95 if sh == -1 else 0
                cc = 95 if sw == -1 else 0
                idx = 8 + (2 if rr == 95 else 0) + (1 if cc == 95 else 0)
                C12[:, idx] += a
    C12 = C12.astype(np.float32)

    pp = np.zeros((NCB, 128, 14), np.float32)
    ppb = np.zeros((NCB, 128, 8), bf16np)
    for cb in range(NCB):
        s = slice(cb * 128, (cb + 1) * 128)
        pp[cb, :, 0:12] = C12[s]
        pp[cb, :, 12] = d[s]
        pp[cb, :, 13] = shift[s]
        ppb[cb] = cls_dense_w[:, s].T.astype(bf16np)

    ke = kernel_embed.reshape(E, C, 9) * inv[None, :, None]
    be = bias_embed * inv[None, :]
    emb = np.zeros((E, NCB * 10 * 128), bf16np)
    for cb in range(NCB):
        s = slice(cb * 128, (cb + 1) * 128)
        for t in range(9):
            emb[:, (cb * 10 + t) * 128:(cb * 10 + t + 1) * 128] = \
                ke[:, s, t].astype(bf16np)
        emb[:, (cb * 10 + 9) * 128:(cb * 10 + 10) * 128] = \
            be[:, s].astype(bf16np)

    db = cls_dense_b.reshape(1, E).astype(np.float32)
    eye = np.eye(128, dtype=np.float32) * 16.0
    id8 = np.concatenate([eye, eye], axis=1).astype(f8np)

    blob = np.zeros((128, BLOB_B), np.uint8)
    for cb in range(NCB):
        blob[:, OFF_PP[cb]:OFF_PP[cb] + 56] = \
            pp[cb].view(np.uint8).reshape(128, 56)
        blob[:, OFF_PPB[cb]:OFF_PPB[cb] + 16] = \
            ppb[cb].view(np.uint8).reshape(128, 16)
    blob[0, OFF_DB:OFF_DB + 32] = db.view(np.uint8).reshape(32)
    blob[:, OFF_ID8:OFF_ID8 + 256] = id8.view(np.uint8)
    blob[0:8, OFF_EMB:BLOB_B] = emb.view(np.uint8).reshape(8, -1)
    return blob


def kernel(x, kernel_embed, bias_embed, cls_conv_w, cls_conv_b,
           cls_bn_gamma, cls_bn_beta, cls_bn_mean, cls_bn_var,
           cls_dense_w, cls_dense_b, bn_gamma, bn_beta, bn_mean, bn_var,
           _trace=False, _trace_kwargs=None):
    from concourse.mybir import dt as _dt
    f8np = _dt.np(f8)

    x = np.asarray(x, dtype=np.float32)
    xp_full = np.zeros((B, C, H, WP), np.float32)
    xp_full[:, :, :, 1:97] = x
    x8_full = xp_full.astype(f8np)
    r8_full = (xp_full - x8_full.astype(np.float32)).astype(f8np)
    args = [np.asarray(a, dtype=np.float32) for a in
            (kernel_embed, bias_embed, cls_conv_w, cls_conv_b,
             cls_bn_gamma, cls_bn_beta, cls_bn_mean, cls_bn_var,
             cls_dense_w, cls_dense_b, bn_gamma, bn_beta, bn_mean, bn_var)]
    blob = _host_prep(*args)

    nc = _get_nc()
    in_maps = []
    for core in range(NCORES):
        s = slice(core * BLOC, (core + 1) * BLOC)
        in_maps.append({"x8": np.ascontiguousarray(x8_full[s]),
                        "r8": np.ascontiguousarray(r8_full[s]),
                        "blob": blob})
    kw = {}
    if _trace:
        kw["trace"] = True
        if _trace_kwargs:
            kw.update(_trace_kwargs)
    res = run_bass_kernel_spmd(nc, in_maps, core_ids=list(range(NCORES)), **kw)
    out = np.concatenate(
        [res.results[i]["out"].astype(np.float32) for i in range(NCORES)],
        axis=0)
    if _trace:
        return out, res
    return out


# revision 54
# speedup vs baseline: 1.2110x; 1.2110x over previous
"""Trainium2 Bass kernel for LowLightAdaptiveCNNBlock (moe_routing).

Full inputs in, full outputs out. Data-parallel over batch on 8 NeuronCores
(2 samples/core), params replicated.

Per-core program (SPMD), v2 — 6-pass tap-paired conv:
  - x shipped as fp8e4m3 pair (x8, r8 = fp8(x - x8)). SBUF planes are
    row-padded: plane = 98 rows x 98 cols, image row r at (r+1)*98, rows 0
    and 97 memset to zero, cols 0/97 zero from host. All conv chunks are
    then uniform -- taps read zeros at the borders exactly like the
    reference's zero padding (no row clipping anywhere).
  - main depthwise 3x3 conv: 6 DoubleRow matmul passes per chunk instead of
    10. Each DR pass contracts TWO tap-shifted views of the x8 plane in one
    matmul by giving the rhs pair dimension an explicit element stride
    (pair stride >= 2 required by HW; stride 1 wedges the device):
      P_res (1,1) x (x8(0,0), r8(0,0))          pair stride PL2
      P0 (k00,k02) x (x8(-1,-1), x8(-1,+1))     pair stride 2
      P1 (k10,k12) x (x8( 0,-1), x8( 0,+1))     pair stride 2
      P2 (k20,k22) x (x8(+1,-1), x8(+1,+1))     pair stride 2
      P3 (k01,k11) x (x8(-1, 0), x8( 0, 0))     pair stride 98
      P4 (k21,k11r) x (x8(+1,0), r8( 0, 0))     pair stride PL2-98
    k11r = center tap applied to r8 (the largest fp8-residual correction);
    the other 8 taps drop their r8 term (small vs the fp8 weight noise).
    BN scale folded into tap weights; epilogue relu(psum + off) -> bf16.
  - routing path via pooled-sum algebra on the x8 plane: mean(BN(dwconv+b))
    is linear in 16 per-channel sums (8 eighth-plane sums, border rows/
    cols, corners). Sample-0 x8 arrives as 16 eighth-DMAs raced on the
    sync (cb0) and gpsimd (cb1) queues; eighth reduces chase arrivals on
    Vector/Scalar/GpSimd round-robin. Consts ride one packed blob DMA on
    the scalar queue.
  - softmax routing weights -> expert kernel/bias mixing via tiny bf16 PE
    matmuls; mixed tap weights written as fp8 diag half-planes from an
    fp8 [I|I] constant (one [128,128] write per tap slot).

Ordering note: cross-engine waits compile to per-engine position counters,
so consumers wait for everything issued earlier on the producer engine.
Sample 1's VectorE routing work is issued only after all of sample 0's
VectorE work and drains under sample 0's conv (ScalarE-only evacuation).
"""
import sys

sys.path.insert(0, "/opt/trn_rl_repo")

import numpy as np
import concourse.bass as bass
import concourse.bacc as bacc
import concourse.tile as tile
from concourse import mybir
from concourse.ap import AP
from concourse.bass_utils import run_bass_kernel_spmd

f32 = mybir.dt.float32
f8 = mybir.dt.float8e4
bf16 = mybir.dt.bfloat16
u8 = mybir.dt.uint8

EPS = 1e-5
B, C, H, W, E = 16, 256, 96, 96, 8
NCORES = 8
BLOC = B // NCORES          # samples per core
NCB = C // 128              # channel blocks
WP = W + 2                  # padded row stride (98)
HP = H + 2                  # padded rows (98)
PL2 = WP * HP               # 9604 = padded plane span
XTD = WP * H                # 9408 = data span (rows 1..96 of plane)
XE = XTD // 8               # 1176 = eighth piece (12 image rows)
ROWCH = 5                   # rows per conv chunk
GROUP = 3                   # chunks per PSUM group
HW = H * W

XQ = XTD // 4               # 2352 = quarter piece (24 image rows)

# const blob byte offsets (all 4-aligned)
OFF_PP = (0, 56)            # per cb: [128, 14] f32
OFF_PPB = (112, 128)        # per cb: [128, 8] bf16
OFF_DB = 144                # [1, 8] f32 (row 0)
OFF_ID8 = 176               # [128, 256] f8
OFF_EMB = 432               # [8, NCB*10*128] bf16 (rows 0..7)
BLOB_B = OFF_EMB + NCB * 10 * 128 * 2   # 5552

CHUNKS = []
_r = 0
while _r < H:
    _nr = min(ROWCH, H - _r)
    CHUNKS.append((_r, _nr))
    _r += _nr

# conv pass table: (lhs half0 tap, lhs half1 tap, rhs base (sh,sw), pair Δ)
# tap index t = (sh+1)*3 + (sw+1); -1 = x8 residual (weight 1). All lhs
# weights are scaled x16 (id8 = 16[I|I]) so small mixed taps stay out of
# fp8 subnormals; the epilogue rescales by 1/16. The r8 residual plus the
# center-tap r8 correction are applied in the epilogue as (16+16*k4)*r8.
PASSES = [
    (0, 2, (-1, -1), 2),
    (3, 5, (0, -1), 2),
    (6, 8, (1, -1), 2),
    (1, 4, (-1, 0), WP),
    (-1, 7, (0, 0), WP),    # (residual x8, k7 @ (+1,0))
]


def _build_program():
    nc = bacc.Bacc("TRN2", target_bir_lowering=False, debug=False,
                   num_devices=NCORES)

    x8_d = nc.dram_tensor("x8", [BLOC, C, H, WP], f8,
                          kind="ExternalInput").ap()
    r8_d = nc.dram_tensor("r8", [BLOC, C, H, WP], f8,
                          kind="ExternalInput").ap()
    blob_d = nc.dram_tensor("blob", [128, BLOB_B], u8,
                            kind="ExternalInput").ap()
    out_d = nc.dram_tensor("out", [BLOC, C, H, W], bf16,
                           kind="ExternalOutput").ap()

    Relu = mybir.ActivationFunctionType.Relu
    Exp = mybir.ActivationFunctionType.Exp
    Copy = mybir.ActivationFunctionType.Copy
    add_op = mybir.AluOpType.add
    mult_op = mybir.AluOpType.mult
    max_op = mybir.AluOpType.max
    AX = mybir.AxisListType.X
    AXY = mybir.AxisListType.XY
    DR = mybir.MatmulPerfMode.DoubleRow

    with tile.TileContext(nc) as tc:
        with tc.tile_pool(name="const", bufs=1) as cpool, \
             tc.tile_pool(name="xp", bufs=4) as xpool, \
             tc.tile_pool(name="small", bufs=4) as spool, \
             tc.tile_pool(name="rp", bufs=1) as rpool, \
             tc.tile_pool(name="dg", bufs=4) as dgpool, \
             tc.tile_pool(name="st", bufs=8) as stpool, \
             tc.tile_pool(name="tm", bufs=8) as tmpool, \
             tc.tile_pool(name="s32", bufs=14) as s32pool, \
             tc.tile_pool(name="sc", bufs=1) as scpool, \
             tc.tile_pool(name="pc", bufs=7, space="PSUM") as pconv, \
             tc.tile_pool(name="pxs", bufs=1, space="PSUM") as psmall:

            xt = [[xpool.tile([128, 2 * PL2], f8, tag="xt",
                              name=f"xt{b}{cb}")
                   for cb in range(NCB)] for b in range(BLOC)]

            # zero the pad rows (rows 0 and 97 of both planes) of the
            # sample-0 tiles. Disjoint from the DMA'd interior, so these
            # overlap the transfers. Sample-1 pads are zeroed later (not
            # needed until its conv) to keep gpsimd's DMA issues early.
            for cb in range(NCB):
                t = xt[0][cb]
                for pl in (0, PL2):
                    nc.vector.memset(t[:, pl:pl + WP], 0.0)
                    nc.vector.memset(t[:, pl + 97 * WP:pl + 98 * WP], 0.0)

            blob_t = cpool.tile([128, BLOB_B], u8, tag="blob")
            pp_t = [blob_t[:, OFF_PP[cb]:OFF_PP[cb] + 56].bitcast(f32)
                    for cb in range(NCB)]
            ppb_t = [blob_t[:, OFF_PPB[cb]:OFF_PPB[cb] + 16].bitcast(bf16)
                     for cb in range(NCB)]
            db_t = blob_t[0:1, OFF_DB:OFF_DB + 32].bitcast(f32)
            id8_t = blob_t[:, OFF_ID8:OFF_ID8 + 256].bitcast(f8)
            emb_t = blob_t[0:8, OFF_EMB:BLOB_B].bitcast(bf16)

            # ---- x8 sample-0 quarter pieces raced on the sync (cb0) and
            # scalar (cb1, behind the blob) HWDGE queues; r8 s0 + all of
            # s1 trail on the gpsimd software queue.
            # The HWDGE queues (sync, scalar) have ~1.5us issue-to-data
            # latency; the gpsimd software queue has ~11us latency but
            # high throughput. Startup-critical x8 sample-0 halves ride
            # the HW queues (3 DMAs each -- more stalls the sem ring);
            # everything else rides gpsimd and lands from ~19us on.
            # x8 s0 halves on the low-latency HW queues (h0 consumed from
            # ~14us, h1 from ~19); everything else on the gpsimd software
            # queue (~11us latency), ordered by need time
            sl_h0 = slice(WP, WP + 2 * XQ)
            sl_h1 = slice(WP + 2 * XQ, WP + 4 * XQ)
            nc.sync.dma_start(xt[0][0][:, sl_h0], x8_d[0, 0:128, 0:48])
            nc.scalar.dma_start(xt[0][1][:, sl_h0], x8_d[0, 128:256, 0:48])
            nc.gpsimd.dma_start(xt[0][0][:, sl_h1], x8_d[0, 0:128, 48:96])
            nc.gpsimd.dma_start(xt[0][1][:, sl_h1], x8_d[0, 128:256, 48:96])
            nc.gpsimd.dma_start(blob_t[:], blob_d)
            for cb in range(NCB):
                s = slice(cb * 128, (cb + 1) * 128)
                nc.gpsimd.dma_start(xt[0][cb][:, PL2 + WP:PL2 + WP + XTD],
                                    r8_d[0, s])
            for cb in range(NCB):
                s = slice(cb * 128, (cb + 1) * 128)
                nc.gpsimd.dma_start(xt[1][cb][:, WP:WP + XTD], x8_d[1, s])
            for cb in range(NCB):
                s = slice(cb * 128, (cb + 1) * 128)
                nc.gpsimd.dma_start(xt[1][cb][:, PL2 + WP:PL2 + WP + XTD],
                                    r8_d[1, s])

            ones_t = cpool.tile([1, 1], bf16, tag="ones")
            nc.vector.memset(ones_t[:], 1.0)
            sink_s = scpool.tile([128, XQ], f8, tag="sinks")

            r_t = [[rpool.tile([128, 1], bf16, tag=f"r{b}{cb}",
                               name=f"r_t{b}{cb}")
                    for cb in range(NCB)] for b in range(BLOC)]
            U_t = [[None] * NCB for _ in range(BLOC)]

            def alloc_U(b, cb):
                U = spool.tile([128, 16], f32, tag=f"U{b}{cb}",
                               name=f"U{b}{cb}")
                U_t[b][cb] = U
                return U

            def reduce_quarter(b, cb, q, eng):
                t = xt[b][cb]
                U = U_t[b][cb]
                src = t[:, WP + q * XQ:WP + (q + 1) * XQ]
                if eng == "vector":
                    nc.vector.tensor_reduce(U[:, q:q + 1], src, axis=AX,
                                            op=add_op)
                else:
                    nc.scalar.activation(sink_s[:], src, Copy,
                                         accum_out=U[:, q:q + 1])

            def small_stats_v(b, cb):
                """border stats on VectorE.
                U layout: [Q0..Q3, R0, R95, C0, C95, x00, x0_95, x95_0,
                x95_95]"""
                t = xt[b][cb]
                U = U_t[b][cb]
                colv = t[:, WP:WP + XTD].rearrange("p (r c) -> p r c", c=WP)
                nc.vector.tensor_reduce(U[:, 4:6], colv[:, 0:96:95, 1:97],
                                        axis=AX, op=add_op)
                nc.vector.tensor_reduce(U[:, 6:7], colv[:, :, 1:2],
                                        axis=AXY, op=add_op)
                nc.vector.tensor_reduce(U[:, 7:8], colv[:, :, 96:97],
                                        axis=AXY, op=add_op)
                nc.vector.tensor_copy(U[:, 8:12], colv[:, 0:96:95, 1:97:95])

            def small_stats_s(b, cb):
                """border stats on ScalarE via activation accumulate."""
                t = xt[b][cb]
                U = U_t[b][cb]
                colv = t[:, WP:WP + XTD].rearrange("p (r c) -> p r c", c=WP)
                snk = sink_s[:, 0:96]
                nc.scalar.activation(snk, colv[:, 0:1, 1:97], Copy,
                                     accum_out=U[:, 4:5])
                nc.scalar.activation(snk, colv[:, 95:96, 1:97], Copy,
                                     accum_out=U[:, 5:6])
                nc.scalar.activation(snk, colv[:, :, 1:2], Copy,
                                     accum_out=U[:, 6:7])
                nc.scalar.activation(snk, colv[:, :, 96:97], Copy,
                                     accum_out=U[:, 7:8])
                nc.scalar.activation(U[:, 8:12], colv[:, 0:96:95, 1:97:95],
                                     Copy)

            def combine_r(b, cb):
                """U * pp -> pooled mean -> relu -> r_t (VectorE)."""
                U = U_t[b][cb]
                scr = spool.tile([128, 12], f32, tag="scr")
                m_t = spool.tile([128, 1], f32, tag="m")
                nc.vector.tensor_tensor(scr[:], U[:, 0:12],
                                        pp_t[cb][:, 0:12], op=mult_op)
                nc.vector.tensor_reduce(m_t[:], scr[:], axis=AX, op=add_op)
                nc.vector.tensor_scalar(r_t[b][cb][:], m_t[:],
                                        pp_t[cb][:, 12:13], 0.0,
                                        op0=add_op, op1=max_op)

            def routing_tail(b):
                pl = psmall.tile([1, E], f32, tag="psm")
                for cb in range(NCB):
                    nc.tensor.matmul(pl[:], r_t[b][cb][:], ppb_t[cb],
                                     start=(cb == 0), stop=(cb == NCB - 1))
                lg = spool.tile([1, E], f32, tag="lg")
                nc.vector.tensor_tensor(lg[:], pl[:], db_t, op=add_op)
                # logits are O(+-1): exp without max-subtraction is safe
                ex = spool.tile([1, E], f32, tag="ex")
                nc.scalar.activation(ex[:], lg[:], Exp, bias=0.0, scale=1.0)
                sm = spool.tile([1, 1], f32, tag="sm")
                nc.vector.reduce_sum(sm[:], ex[:], axis=AX)
                rs = spool.tile([1, 1], f32, tag="rs")
                nc.vector.reciprocal(rs[:], sm[:])
                wsm = spool.tile([1, E], bf16, tag="wsm")
                nc.vector.tensor_scalar_mul(wsm[:], ex[:], rs[:])
                pw = psmall.tile([E, 1], f32, tag="psm")
                nc.tensor.matmul(pw[:], wsm[:], ones_t[:], start=True,
                                 stop=True)
                wT = spool.tile([E, 1], bf16, tag="wT")
                nc.vector.tensor_copy(wT[:], pw[:])
                return wT

            def emb_sl(cb, t):
                return emb_t[:, (cb * 10 + t) * 128:(cb * 10 + t + 1) * 128]

            def mix_kb(b, cb, wT, diag_engine):
                """mixed+scaled tap weights -> fp8 diag half planes
                [128, 10*128] (5 DR pairs) and off [128, 1]."""
                pk = psmall.tile([128, 10], f32, tag="psm")
                for t in range(10):
                    nc.tensor.matmul(pk[:, t:t + 1], emb_sl(cb, t), wT[:],
                                     start=True, stop=True)
                kb = spool.tile([128, 10], f32, tag="kb")
                nc.vector.tensor_copy(kb[:], pk[:])
                off = spool.tile([128, 1], f32, tag="off")
                nc.vector.tensor_tensor(off[:], kb[:, 9:10],
                                        pp_t[cb][:, 13:14], op=add_op)
                diag = dgpool.tile([128, 10 * 128], f8, tag="diag")

                def eng_for(i):
                    if diag_engine == "vector":
                        return nc.vector
                    if diag_engine == "scalar":
                        return nc.scalar
                    if diag_engine == "tri":
                        return (nc.vector, nc.scalar, nc.gpsimd)[i % 3]
                    return nc.vector if i % 2 == 0 else nc.scalar

                half_taps = []
                for (ta, tb, _, _) in PASSES:
                    half_taps += [ta, tb]
                id1 = id8_t[:, 0:128]
                for i, t in enumerate(half_taps):
                    dst = diag[:, i * 128:(i + 1) * 128]
                    sc = 1.0 if t < 0 else kb[:, t:t + 1]
                    eng = eng_for(i)
                    if eng is nc.scalar:
                        nc.scalar.activation(dst, id1, Copy, bias=0.0,
                                             scale=sc)
                    else:
                        eng.tensor_scalar_mul(dst, id1, sc)
                # epilogue r8 coefficient: (1 + k4) * 16 per channel
                k4p = spool.tile([128, 1], f32, tag="k4p")
                nc.vector.tensor_scalar(k4p[:], kb[:, 4:5], 1.0, 16.0,
                                        op0=add_op, op1=mult_op)
                return diag, off, k4p

            def pair_rhs(t, base, delta, nr):
                return AP(t.tensor, t.offset + base,
                          [list(t.ap[0]), [delta, 2], [WP, nr], [1, 96]])

            def conv_tile(b, cb, diag, off, k4p):
                t = xt[b][cb]

                ng = (len(CHUNKS) + GROUP - 1) // GROUP
                for g in range(ng):
                    gchunks = CHUNKS[g * GROUP:(g + 1) * GROUP]
                    pss = []
                    for ci in range(len(gchunks)):
                        pss.append(pconv.tile([128, ROWCH * W], f32,
                                              tag="pc", name=f"ps{ci}"))
                    # r8 * (16 + 16*k4) on GpSimd, ahead of the matmul stops
                    tmps = []
                    for ci, (r0, nr) in enumerate(gchunks):
                        n = nr * W
                        tmp = tmpool.tile([128, ROWCH * W], f32, tag="tmp")
                        r8v = AP(t.tensor,
                                 t.offset + PL2 + WP * (r0 + 1) + 1,
                                 [list(t.ap[0]), [WP, nr], [1, 96]])
                        nc.gpsimd.tensor_scalar(tmp[:, 0:n], r8v,
                                                k4p[:, 0:1], 0.0,
                                                op0=mult_op, op1=add_op)
                        tmps.append(tmp)
                    for pi, (ta, tb, (sh, sw), delta) in enumerate(PASSES):
                        lhs = diag[:, 2 * pi * 128:
                                   (2 * pi + 2) * 128].rearrange(
                                       "p (two m) -> p two m", two=2)
                        for ci, (r0, nr) in enumerate(gchunks):
                            base = WP * (r0 + sh + 1) + 1 + sw
                            nc.tensor.matmul(
                                pss[ci][:, 0:nr * W], lhs,
                                pair_rhs(t, base, delta, nr),
                                start=(pi == 0),
                                stop=(pi == len(PASSES) - 1),
                                perf_mode=DR)
                    for ci, (r0, nr) in enumerate(gchunks):
                        n = nr * W
                        st32 = s32pool.tile([128, ROWCH * W], f32,
                                            tag="st32")
                        nc.vector.tensor_tensor(st32[:, 0:n],
                                                pss[ci][:, 0:n],
                                                tmps[ci][:, 0:n], op=add_op)
                        stage = stpool.tile([128, ROWCH * W], bf16, tag="st")
                        nc.scalar.activation(stage[:, 0:n], st32[:, 0:n],
                                             Relu, bias=off[:],
                                             scale=0.0625)
                        nc.sync.dma_start(
                            out_d[b, cb * 128:(cb + 1) * 128, r0:r0 + nr],
                            stage[:, 0:n])

            # ---- sample-0 routing: quarter reduces chase the DMA arrivals
            for cb in range(NCB):
                alloc_U(0, cb)
            ENG2 = ("vector", "scalar")
            for q in range(4):
                for cb in range(NCB):
                    reduce_quarter(0, cb, q, ENG2[(q + cb) % 2])
            small_stats_v(0, 0)
            small_stats_v(0, 1)
            combine_r(0, 0)
            combine_r(0, 1)

            wT0 = routing_tail(0)
            dg00 = mix_kb(0, 0, wT0, "split")
            dg01 = mix_kb(0, 1, wT0, "scalar")

            # sample-1 routing on VectorE only, held back so the static
            # scheduler doesn't weave it into sample 0's startup chain
            with tc.tile_wait_until(0.022):
                for cb in range(NCB):
                    alloc_U(1, cb)
                # sample-1 sums on ScalarE (VectorE is busy evacuating
                # PSUM for the 5-pass epilogue; the deep st32 pool absorbs
                # ScalarE's pause in the relu/stage stream)
                for q in range(4):
                    for cb in range(NCB):
                        reduce_quarter(1, cb, q, "scalar")
                small_stats_v(1, 0)
                small_stats_v(1, 1)
                combine_r(1, 0)
                combine_r(1, 1)

            # sample-1 pad rows (gpsimd, after its DMA issues)
            for cb in range(NCB):
                t = xt[1][cb]
                for pl in (0, PL2):
                    nc.gpsimd.memset(t[:, pl:pl + WP], 0.0)
                    nc.gpsimd.memset(t[:, pl + 97 * WP:pl + 98 * WP], 0.0)

            conv_tile(0, 0, *dg00)

            wT1 = routing_tail(1)
            dg10 = mix_kb(1, 0, wT1, "vector")
            dg11 = mix_kb(1, 1, wT1, "vector")

            conv_tile(0, 1, *dg01)
            conv_tile(1, 0, *dg10)
            conv_tile(1, 1, *dg11)

    if not nc.is_finalized():
        nc.finalize()
    return nc


_NC = None


def _get_nc():
    global _NC
    if _NC is None:
        _NC = _build_program()
    return _NC


def _host_prep(kernel_embed, bias_embed, cls_conv_w, cls_conv_b,
               cls_bn_gamma, cls_bn_beta, cls_bn_mean, cls_bn_var,
               cls_dense_w, cls_dense_b, bn_gamma, bn_beta, bn_mean, bn_var):
    from concourse.mybir import dt as _dt
    bf16np = _dt.np(bf16)
    f8np = _dt.np(f8)

    inv = bn_gamma / np.sqrt(bn_var + EPS)
    shift = bn_beta - bn_mean * inv
    cls_inv = cls_bn_gamma / np.sqrt(cls_bn_var + EPS)
    cls_shift = cls_bn_beta - cls_bn_mean * cls_inv

    cls_w9 = cls_conv_w.reshape(C, 9)
    A = (cls_inv[:, None] * cls_w9) / HW                      # (C, 9)
    d = cls_inv * cls_conv_b + cls_shift                      # (C,)

    # window-sum decomposition: S_ij = T - rho - gamma + kappa
    # stat order: [Q0..Q3, R0, R95, C0, C95, x00, x0_95, x95_0, x95_95]
    C12 = np.zeros((C, 12), np.float64)
    for i, sh in enumerate((-1, 0, 1)):
        for j, sw in enumerate((-1, 0, 1)):
            a = A[:, i * 3 + j].astype(np.float64)
            for q in range(4):
                C12[:, q] += a
            if sh == -1:
                C12[:, 5] -= a
            elif sh == 1:
                C12[:, 4] -= a
            if sw == -1:
                C12[:, 7] -= a
            elif sw == 1:
                C12[:, 6] -= a
            if sh != 0 and sw != 0:
                rr = 95 if sh == -1 else 0
                cc = 95 if sw == -1 else 0
                idx = 8 + (2 if rr == 95 else 0) + (1 if cc == 95 else 0)
                C12[:, idx] += a
    C12 = C12.astype(np.float32)

    pp = np.zeros((NCB, 128, 14), np.float32)
    ppb = np.zeros((NCB, 128, 8), bf16np)
    for cb in range(NCB):
        s = slice(cb * 128, (cb + 1) * 128)
        pp[cb, :, 0:12] = C12[s]
        pp[cb, :, 12] = d[s]
        pp[cb, :, 13] = shift[s]
        ppb[cb] = cls_dense_w[:, s].T.astype(bf16np)

    ke = kernel_embed.reshape(E, C, 9) * inv[None, :, None]
    be = bias_embed * inv[None, :]
    emb = np.zeros((E, NCB * 10 * 128), bf16np)
    for cb in range(NCB):
        s = slice(cb * 128, (cb + 1) * 128)
        for t in range(9):
            emb[:, (cb * 10 + t) * 128:(cb * 10 + t + 1) * 128] = \
                ke[:, s, t].astype(bf16np)
        emb[:, (cb * 10 + 9) * 128:(cb * 10 + 10) * 128] = \
            be[:, s].astype(bf16np)

    db = cls_dense_b.reshape(1, E).astype(np.float32)
    eye = np.eye(128, dtype=np.float32) * 16.0
    id8 = np.concatenate([eye, eye], axis=1).astype(f8np)

    blob = np.zeros((128, BLOB_B), np.uint8)
    for cb in range(NCB):
        blob[:, OFF_PP[cb]:OFF_PP[cb] + 56] = \
            pp[cb].view(np.uint8).reshape(128, 56)
        blob[:, OFF_PPB[cb]:OFF_PPB[cb] + 16] = \
            ppb[cb].view(np.uint8).reshape(128, 16)
    blob[0, OFF_DB:OFF_DB + 32] = db.view(np.uint8).reshape(32)
    blob[:, OFF_ID8:OFF_ID8 + 256] = id8.view(np.uint8)
    blob[0:8, OFF_EMB:BLOB_B] = emb.view(np.uint8).reshape(8, -1)
    return blob


def kernel(x, kernel_embed, bias_embed, cls_conv_w, cls_conv_b,
           cls_bn_gamma, cls_bn_beta, cls_bn_mean, cls_bn_var,
           cls_dense_w, cls_dense_b, bn_gamma, bn_beta, bn_mean, bn_var,
           _trace=False, _trace_kwargs=None):
    from concourse.mybir import dt as _dt
    f8np = _dt.np(f8)

    x = np.asarray(x, dtype=np.float32)
    xp_full = np.zeros((B, C, H, WP), np.float32)
    xp_full[:, :, :, 1:97] = x
    x8_full = xp_full.astype(f8np)
    r8_full = (xp_full - x8_full.astype(np.float32)).astype(f8np)
    args = [np.asarray(a, dtype=np.float32) for a in
            (kernel_embed, bias_embed, cls_conv_w, cls_conv_b,
             cls_bn_gamma, cls_bn_beta, cls_bn_mean, cls_bn_var,
             cls_dense_w, cls_dense_b, bn_gamma, bn_beta, bn_mean, bn_var)]
    blob = _host_prep(*args)

    nc = _get_nc()
    in_maps = []
    for core in range(NCORES):
        s = slice(core * BLOC, (core + 1) * BLOC)
        in_maps.append({"x8": np.ascontiguousarray(x8_full[s]),
                        "r8": np.ascontiguousarray(r8_full[s]),
                        "blob": blob})
    kw = {}
    if _trace:
        kw["trace"] = True
        if _trace_kwargs:
            kw.update(_trace_kwargs)
    res = run_bass_kernel_spmd(nc, in_maps, core_ids=list(range(NCORES)), **kw)
    out = np.concatenate(
        [res.results[i]["out"].astype(np.float32) for i in range(NCORES)],
        axis=0)
    if _trace:
        return out, res
    return out


# revision 56
# speedup vs baseline: 1.2133x; 1.0019x over previous
"""Trainium2 Bass kernel for LowLightAdaptiveCNNBlock (moe_routing).

Full inputs in, full outputs out. Data-parallel over batch on 8 NeuronCores
(2 samples/core), params replicated.

Per-core program (SPMD), v2 — 6-pass tap-paired conv:
  - x shipped as fp8e4m3 pair (x8, r8 = fp8(x - x8)). SBUF planes are
    row-padded: plane = 98 rows x 98 cols, image row r at (r+1)*98, rows 0
    and 97 memset to zero, cols 0/97 zero from host. All conv chunks are
    then uniform -- taps read zeros at the borders exactly like the
    reference's zero padding (no row clipping anywhere).
  - main depthwise 3x3 conv: 6 DoubleRow matmul passes per chunk instead of
    10. Each DR pass contracts TWO tap-shifted views of the x8 plane in one
    matmul by giving the rhs pair dimension an explicit element stride
    (pair stride >= 2 required by HW; stride 1 wedges the device):
      P_res (1,1) x (x8(0,0), r8(0,0))          pair stride PL2
      P0 (k00,k02) x (x8(-1,-1), x8(-1,+1))     pair stride 2
      P1 (k10,k12) x (x8( 0,-1), x8( 0,+1))     pair stride 2
      P2 (k20,k22) x (x8(+1,-1), x8(+1,+1))     pair stride 2
      P3 (k01,k11) x (x8(-1, 0), x8( 0, 0))     pair stride 98
      P4 (k21,k11r) x (x8(+1,0), r8( 0, 0))     pair stride PL2-98
    k11r = center tap applied to r8 (the largest fp8-residual correction);
    the other 8 taps drop their r8 term (small vs the fp8 weight noise).
    BN scale folded into tap weights; epilogue relu(psum + off) -> bf16.
  - routing path via pooled-sum algebra on the x8 plane: mean(BN(dwconv+b))
    is linear in 16 per-channel sums (8 eighth-plane sums, border rows/
    cols, corners). Sample-0 x8 arrives as 16 eighth-DMAs raced on the
    sync (cb0) and gpsimd (cb1) queues; eighth reduces chase arrivals on
    Vector/Scalar/GpSimd round-robin. Consts ride one packed blob DMA on
    the scalar queue.
  - softmax routing weights -> expert kernel/bias mixing via tiny bf16 PE
    matmuls; mixed tap weights written as fp8 diag half-planes from an
    fp8 [I|I] constant (one [128,128] write per tap slot).

Ordering note: cross-engine waits compile to per-engine position counters,
so consumers wait for everything issued earlier on the producer engine.
Sample 1's VectorE routing work is issued only after all of sample 0's
VectorE work and drains under sample 0's conv (ScalarE-only evacuation).
"""
import sys

sys.path.insert(0, "/opt/trn_rl_repo")

import numpy as np
import concourse.bass as bass
import concourse.bacc as bacc
import concourse.tile as tile
from concourse import mybir
from concourse.ap import AP
from concourse.bass_utils import run_bass_kernel_spmd

f32 = mybir.dt.float32
f8 = mybir.dt.float8e4
bf16 = mybir.dt.bfloat16
u8 = mybir.dt.uint8

EPS = 1e-5
B, C, H, W, E = 16, 256, 96, 96, 8
NCORES = 8
BLOC = B // NCORES          # samples per core
NCB = C // 128              # channel blocks
WP = W + 2                  # padded row stride (98)
HP = H + 2                  # padded rows (98)
PL2 = WP * HP               # 9604 = padded plane span
XTD = WP * H                # 9408 = data span (rows 1..96 of plane)
XE = XTD // 8               # 1176 = eighth piece (12 image rows)
ROWCH = 5                   # rows per conv chunk
GROUP = 3                   # chunks per PSUM group
HW = H * W

XQ = XTD // 4               # 2352 = quarter piece (24 image rows)

# const blob byte offsets (all 4-aligned)
OFF_PP = (0, 56)            # per cb: [128, 14] f32
OFF_PPB = (112, 128)        # per cb: [128, 8] bf16
OFF_DB = 144                # [1, 8] f32 (row 0)
OFF_ID8 = 176               # [128, 256] f8
OFF_EMB = 432               # [8, NCB*10*128] bf16 (rows 0..7)
BLOB_B = OFF_EMB + NCB * 10 * 128 * 2   # 5552

CHUNKS = []
_r = 0
while _r < H:
    _nr = min(ROWCH, H - _r)
    CHUNKS.append((_r, _nr))
    _r += _nr

# conv pass table: (lhs half0 tap, lhs half1 tap, rhs base (sh,sw), pair Δ)
# tap index t = (sh+1)*3 + (sw+1); -1 = x8 residual (weight 1). All lhs
# weights are scaled x16 (id8 = 16[I|I]) so small mixed taps stay out of
# fp8 subnormals; the epilogue rescales by 1/16. The r8 residual plus the
# center-tap r8 correction are applied in the epilogue as (16+16*k4)*r8.
PASSES = [
    (0, 2, (-1, -1), 2),
    (3, 5, (0, -1), 2),
    (6, 8, (1, -1), 2),
    (1, 4, (-1, 0), WP),
    (-1, 7, (0, 0), WP),    # (residual x8, k7 @ (+1,0))
]


def _build_program():
    nc = bacc.Bacc("TRN2", target_bir_lowering=False, debug=False,
                   num_devices=NCORES)

    x8_d = nc.dram_tensor("x8", [BLOC, C, H, WP], f8,
                          kind="ExternalInput").ap()
    r8_d = nc.dram_tensor("r8", [BLOC, C, H, WP], f8,
                          kind="ExternalInput").ap()
    blob_d = nc.dram_tensor("blob", [128, BLOB_B], u8,
                            kind="ExternalInput").ap()
    out_d = nc.dram_tensor("out", [BLOC, C, H, W], bf16,
                           kind="ExternalOutput").ap()

    Relu = mybir.ActivationFunctionType.Relu
    Exp = mybir.ActivationFunctionType.Exp
    Copy = mybir.ActivationFunctionType.Copy
    add_op = mybir.AluOpType.add
    mult_op = mybir.AluOpType.mult
    max_op = mybir.AluOpType.max
    AX = mybir.AxisListType.X
    AXY = mybir.AxisListType.XY
    DR = mybir.MatmulPerfMode.DoubleRow

    with tile.TileContext(nc) as tc:
        with tc.tile_pool(name="const", bufs=1) as cpool, \
             tc.tile_pool(name="xp", bufs=4) as xpool, \
             tc.tile_pool(name="small", bufs=4) as spool, \
             tc.tile_pool(name="rp", bufs=1) as rpool, \
             tc.tile_pool(name="dg", bufs=4) as dgpool, \
             tc.tile_pool(name="st", bufs=8) as stpool, \
             tc.tile_pool(name="tm", bufs=8) as tmpool, \
             tc.tile_pool(name="s32", bufs=14) as s32pool, \
             tc.tile_pool(name="sc", bufs=1) as scpool, \
             tc.tile_pool(name="pc", bufs=7, space="PSUM") as pconv, \
             tc.tile_pool(name="pxs", bufs=1, space="PSUM") as psmall:

            xt = [[xpool.tile([128, 2 * PL2], f8, tag="xt",
                              name=f"xt{b}{cb}")
                   for cb in range(NCB)] for b in range(BLOC)]

            # zero the pad rows (rows 0 and 97 of both planes) of the
            # sample-0 tiles. Disjoint from the DMA'd interior, so these
            # overlap the transfers. Sample-1 pads are zeroed later (not
            # needed until its conv) to keep gpsimd's DMA issues early.
            for cb in range(NCB):
                t = xt[0][cb]
                for pl in (0, PL2):
                    nc.vector.memset(t[:, pl:pl + WP], 0.0)
                    nc.vector.memset(t[:, pl + 97 * WP:pl + 98 * WP], 0.0)

            blob_t = cpool.tile([128, BLOB_B], u8, tag="blob")
            pp_t = [blob_t[:, OFF_PP[cb]:OFF_PP[cb] + 56].bitcast(f32)
                    for cb in range(NCB)]
            ppb_t = [blob_t[:, OFF_PPB[cb]:OFF_PPB[cb] + 16].bitcast(bf16)
                     for cb in range(NCB)]
            db_t = blob_t[0:1, OFF_DB:OFF_DB + 32].bitcast(f32)
            id8_t = blob_t[:, OFF_ID8:OFF_ID8 + 256].bitcast(f8)
            emb_t = blob_t[0:8, OFF_EMB:BLOB_B].bitcast(bf16)

            # ---- x8 sample-0 quarter pieces raced on the sync (cb0) and
            # scalar (cb1, behind the blob) HWDGE queues; r8 s0 + all of
            # s1 trail on the gpsimd software queue.
            # The HWDGE queues (sync, scalar) have ~1.5us issue-to-data
            # latency; the gpsimd software queue has ~11us latency but
            # high throughput. Startup-critical x8 sample-0 halves ride
            # the HW queues (3 DMAs each -- more stalls the sem ring);
            # everything else rides gpsimd and lands from ~19us on.
            # x8 s0 halves on the low-latency HW queues (h0 consumed from
            # ~14us, h1 from ~19); everything else on the gpsimd software
            # queue (~11us latency), ordered by need time
            sl_h0 = slice(WP, WP + 2 * XQ)
            sl_h1 = slice(WP + 2 * XQ, WP + 4 * XQ)
            nc.sync.dma_start(xt[0][0][:, sl_h0], x8_d[0, 0:128, 0:48])
            nc.scalar.dma_start(xt[0][1][:, sl_h0], x8_d[0, 128:256, 0:48])
            nc.gpsimd.dma_start(xt[0][0][:, sl_h1], x8_d[0, 0:128, 48:96])
            nc.gpsimd.dma_start(xt[0][1][:, sl_h1], x8_d[0, 128:256, 48:96])
            nc.gpsimd.dma_start(blob_t[:], blob_d)
            for cb in range(NCB):
                s = slice(cb * 128, (cb + 1) * 128)
                nc.gpsimd.dma_start(xt[0][cb][:, PL2 + WP:PL2 + WP + XTD],
                                    r8_d[0, s])
            for cb in range(NCB):
                s = slice(cb * 128, (cb + 1) * 128)
                nc.gpsimd.dma_start(xt[1][cb][:, WP:WP + XTD], x8_d[1, s])
            for cb in range(NCB):
                s = slice(cb * 128, (cb + 1) * 128)
                nc.gpsimd.dma_start(xt[1][cb][:, PL2 + WP:PL2 + WP + XTD],
                                    r8_d[1, s])

            ones_t = cpool.tile([1, 1], bf16, tag="ones")
            nc.vector.memset(ones_t[:], 1.0)
            sink_s = scpool.tile([128, XQ], f8, tag="sinks")

            r_t = [[rpool.tile([128, 1], bf16, tag=f"r{b}{cb}",
                               name=f"r_t{b}{cb}")
                    for cb in range(NCB)] for b in range(BLOC)]
            U_t = [[None] * NCB for _ in range(BLOC)]

            def alloc_U(b, cb):
                U = spool.tile([128, 16], f32, tag=f"U{b}{cb}",
                               name=f"U{b}{cb}")
                U_t[b][cb] = U
                return U

            def reduce_quarter(b, cb, q, eng):
                t = xt[b][cb]
                U = U_t[b][cb]
                src = t[:, WP + q * XQ:WP + (q + 1) * XQ]
                if eng == "vector":
                    nc.vector.tensor_reduce(U[:, q:q + 1], src, axis=AX,
                                            op=add_op)
                else:
                    nc.scalar.activation(sink_s[:], src, Copy,
                                         accum_out=U[:, q:q + 1])

            def small_stats_v(b, cb):
                """border stats on VectorE.
                U layout: [Q0..Q3, R0, R95, C0, C95, x00, x0_95, x95_0,
                x95_95]"""
                t = xt[b][cb]
                U = U_t[b][cb]
                colv = t[:, WP:WP + XTD].rearrange("p (r c) -> p r c", c=WP)
                nc.vector.tensor_reduce(U[:, 4:6], colv[:, 0:96:95, 1:97],
                                        axis=AX, op=add_op)
                nc.vector.tensor_reduce(U[:, 6:7], colv[:, :, 1:2],
                                        axis=AXY, op=add_op)
                nc.vector.tensor_reduce(U[:, 7:8], colv[:, :, 96:97],
                                        axis=AXY, op=add_op)
                nc.vector.tensor_copy(U[:, 8:12], colv[:, 0:96:95, 1:97:95])

            def small_stats_s(b, cb):
                """border stats on ScalarE via activation accumulate."""
                t = xt[b][cb]
                U = U_t[b][cb]
                colv = t[:, WP:WP + XTD].rearrange("p (r c) -> p r c", c=WP)
                snk = sink_s[:, 0:96]
                nc.scalar.activation(snk, colv[:, 0:1, 1:97], Copy,
                                     accum_out=U[:, 4:5])
                nc.scalar.activation(snk, colv[:, 95:96, 1:97], Copy,
                                     accum_out=U[:, 5:6])
                nc.scalar.activation(snk, colv[:, :, 1:2], Copy,
                                     accum_out=U[:, 6:7])
                nc.scalar.activation(snk, colv[:, :, 96:97], Copy,
                                     accum_out=U[:, 7:8])
                nc.scalar.activation(U[:, 8:12], colv[:, 0:96:95, 1:97:95],
                                     Copy)

            def combine_r(b, cb):
                """U * pp -> pooled mean -> relu -> r_t (VectorE)."""
                U = U_t[b][cb]
                scr = spool.tile([128, 12], f32, tag="scr")
                m_t = spool.tile([128, 1], f32, tag="m")
                nc.vector.tensor_tensor(scr[:], U[:, 0:12],
                                        pp_t[cb][:, 0:12], op=mult_op)
                nc.vector.tensor_reduce(m_t[:], scr[:], axis=AX, op=add_op)
                nc.vector.tensor_scalar(r_t[b][cb][:], m_t[:],
                                        pp_t[cb][:, 12:13], 0.0,
                                        op0=add_op, op1=max_op)

            def routing_tail(b):
                pl = psmall.tile([1, E], f32, tag="psm")
                for cb in range(NCB):
                    nc.tensor.matmul(pl[:], r_t[b][cb][:], ppb_t[cb],
                                     start=(cb == 0), stop=(cb == NCB - 1))
                lg = spool.tile([1, E], f32, tag="lg")
                nc.vector.tensor_tensor(lg[:], pl[:], db_t, op=add_op)
                # logits are O(+-1): exp without max-subtraction is safe
                ex = spool.tile([1, E], f32, tag="ex")
                nc.scalar.activation(ex[:], lg[:], Exp, bias=0.0, scale=1.0)
                sm = spool.tile([1, 1], f32, tag="sm")
                nc.vector.reduce_sum(sm[:], ex[:], axis=AX)
                rs = spool.tile([1, 1], f32, tag="rs")
                nc.vector.reciprocal(rs[:], sm[:])
                wsm = spool.tile([1, E], bf16, tag="wsm")
                nc.vector.tensor_scalar_mul(wsm[:], ex[:], rs[:])
                pw = psmall.tile([E, 1], f32, tag="psm")
                nc.tensor.matmul(pw[:], wsm[:], ones_t[:], start=True,
                                 stop=True)
                wT = spool.tile([E, 1], bf16, tag="wT")
                nc.vector.tensor_copy(wT[:], pw[:])
                return wT

            def emb_sl(cb, t):
                return emb_t[:, (cb * 10 + t) * 128:(cb * 10 + t + 1) * 128]

            def mix_kb(b, cb, wT, diag_engine):
                """mixed+scaled tap weights -> fp8 diag half planes
                [128, 10*128] (5 DR pairs) and off [128, 1]."""
                pk = psmall.tile([128, 10], f32, tag="psm")
                for t in range(10):
                    nc.tensor.matmul(pk[:, t:t + 1], emb_sl(cb, t), wT[:],
                                     start=True, stop=True)
                kb = spool.tile([128, 10], f32, tag="kb")
                nc.vector.tensor_copy(kb[:], pk[:])
                off = spool.tile([128, 1], f32, tag="off")
                nc.vector.tensor_tensor(off[:], kb[:, 9:10],
                                        pp_t[cb][:, 13:14], op=add_op)
                diag = dgpool.tile([128, 10 * 128], f8, tag="diag")

                def eng_for(i):
                    if diag_engine == "vector":
                        return nc.vector
                    if diag_engine == "scalar":
                        return nc.scalar
                    if diag_engine == "tri":
                        return (nc.vector, nc.scalar, nc.gpsimd)[i % 3]
                    return nc.vector if i % 2 == 0 else nc.scalar

                half_taps = []
                for (ta, tb, _, _) in PASSES:
                    half_taps += [ta, tb]
                id1 = id8_t[:, 0:128]
                for i, t in enumerate(half_taps):
                    dst = diag[:, i * 128:(i + 1) * 128]
                    sc = 1.0 if t < 0 else kb[:, t:t + 1]
                    eng = eng_for(i)
                    if eng is nc.scalar:
                        nc.scalar.activation(dst, id1, Copy, bias=0.0,
                                             scale=sc)
                    else:
                        eng.tensor_scalar_mul(dst, id1, sc)
                # epilogue r8 coefficient: (1 + k4) * 16 per channel
                k4p = spool.tile([128, 1], f32, tag="k4p")
                nc.vector.tensor_scalar(k4p[:], kb[:, 4:5], 1.0, 16.0,
                                        op0=add_op, op1=mult_op)
                return diag, off, k4p

            def pair_rhs(t, base, delta, nr):
                return AP(t.tensor, t.offset + base,
                          [list(t.ap[0]), [delta, 2], [WP, nr], [1, 96]])

            def conv_tile(b, cb, diag, off, k4p, final=False):
                t = xt[b][cb]

                groups = [CHUNKS[i:i + GROUP]
                          for i in range(0, len(CHUNKS), GROUP)]
                if final:
                    # end on a solo 1-row chunk so the post-matmul
                    # TS/TT/ACT/DMA drain chain is as short as possible
                    groups = groups[:-1] + [[CHUNKS[18]], [CHUNKS[19]]]
                for g, gchunks in enumerate(groups):
                    pss = []
                    for ci in range(len(gchunks)):
                        pss.append(pconv.tile([128, ROWCH * W], f32,
                                              tag="pc", name=f"ps{ci}"))
                    # r8 * (16 + 16*k4) on GpSimd, ahead of the matmul stops
                    tmps = []
                    for ci, (r0, nr) in enumerate(gchunks):
                        n = nr * W
                        tmp = tmpool.tile([128, ROWCH * W], f32, tag="tmp")
                        r8v = AP(t.tensor,
                                 t.offset + PL2 + WP * (r0 + 1) + 1,
                                 [list(t.ap[0]), [WP, nr], [1, 96]])
                        nc.gpsimd.tensor_scalar(tmp[:, 0:n], r8v,
                                                k4p[:, 0:1], 0.0,
                                                op0=mult_op, op1=add_op)
                        tmps.append(tmp)
                    for pi, (ta, tb, (sh, sw), delta) in enumerate(PASSES):
                        lhs = diag[:, 2 * pi * 128:
                                   (2 * pi + 2) * 128].rearrange(
                                       "p (two m) -> p two m", two=2)
                        for ci, (r0, nr) in enumerate(gchunks):
                            base = WP * (r0 + sh + 1) + 1 + sw
                            nc.tensor.matmul(
                                pss[ci][:, 0:nr * W], lhs,
                                pair_rhs(t, base, delta, nr),
                                start=(pi == 0),
                                stop=(pi == len(PASSES) - 1),
                                perf_mode=DR)
                    for ci, (r0, nr) in enumerate(gchunks):
                        n = nr * W
                        st32 = s32pool.tile([128, ROWCH * W], f32,
                                            tag="st32")
                        nc.vector.tensor_tensor(st32[:, 0:n],
                                                pss[ci][:, 0:n],
                                                tmps[ci][:, 0:n], op=add_op)
                        stage = stpool.tile([128, ROWCH * W], bf16, tag="st")
                        nc.scalar.activation(stage[:, 0:n], st32[:, 0:n],
                                             Relu, bias=off[:],
                                             scale=0.0625)
                        nc.sync.dma_start(
                            out_d[b, cb * 128:(cb + 1) * 128, r0:r0 + nr],
                            stage[:, 0:n])

            # ---- sample-0 routing: quarter reduces chase the DMA arrivals
            for cb in range(NCB):
                alloc_U(0, cb)
            ENG2 = ("vector", "scalar")
            for q in range(4):
                for cb in range(NCB):
                    reduce_quarter(0, cb, q, ENG2[(q + cb) % 2])
            small_stats_v(0, 0)
            small_stats_v(0, 1)
            combine_r(0, 0)
            combine_r(0, 1)

            wT0 = routing_tail(0)
            dg00 = mix_kb(0, 0, wT0, "split")
            dg01 = mix_kb(0, 1, wT0, "scalar")

            # sample-1 routing on VectorE only, held back so the static
            # scheduler doesn't weave it into sample 0's startup chain
            with tc.tile_wait_until(0.022):
                for cb in range(NCB):
                    alloc_U(1, cb)
                # sample-1 sums on ScalarE (VectorE is busy evacuating
                # PSUM for the 5-pass epilogue; the deep st32 pool absorbs
                # ScalarE's pause in the relu/stage stream)
                for q in range(4):
                    for cb in range(NCB):
                        reduce_quarter(1, cb, q, "scalar")
                small_stats_v(1, 0)
                small_stats_v(1, 1)
                combine_r(1, 0)
                combine_r(1, 1)

            # sample-1 pad rows (gpsimd, after its DMA issues)
            for cb in range(NCB):
                t = xt[1][cb]
                for pl in (0, PL2):
                    nc.gpsimd.memset(t[:, pl:pl + WP], 0.0)
                    nc.gpsimd.memset(t[:, pl + 97 * WP:pl + 98 * WP], 0.0)

            conv_tile(0, 0, *dg00)

            wT1 = routing_tail(1)
            dg10 = mix_kb(1, 0, wT1, "vector")
            dg11 = mix_kb(1, 1, wT1, "vector")

            conv_tile(0, 1, *dg01)
            conv_tile(1, 0, *dg10)
            conv_tile(1, 1, *dg11, final=True)

    if not nc.is_finalized():
        nc.finalize()
    return nc


_NC = None


def _get_nc():
    global _NC
    if _NC is None:
        _NC = _build_program()
    return _NC


def _host_prep(kernel_embed, bias_embed, cls_conv_w, cls_conv_b,
               cls_bn_gamma, cls_bn_beta, cls_bn_mean, cls_bn_var,
               cls_dense_w, cls_dense_b, bn_gamma, bn_beta, bn_mean, bn_var):
    from concourse.mybir import dt as _dt
    bf16np = _dt.np(bf16)
    f8np = _dt.np(f8)

    inv = bn_gamma / np.sqrt(bn_var + EPS)
    shift = bn_beta - bn_mean * inv
    cls_inv = cls_bn_gamma / np.sqrt(cls_bn_var + EPS)
    cls_shift = cls_bn_beta - cls_bn_mean * cls_inv

    cls_w9 = cls_conv_w.reshape(C, 9)
    A = (cls_inv[:, None] * cls_w9) / HW                      # (C, 9)
    d = cls_inv * cls_conv_b + cls_shift                      # (C,)

    # window-sum decomposition: S_ij = T - rho - gamma + kappa
    # stat order: [Q0..Q3, R0, R95, C0, C95, x00, x0_95, x95_0, x95_95]
    C12 = np.zeros((C, 12), np.float64)
    for i, sh in enumerate((-1, 0, 1)):
        for j, sw in enumerate((-1, 0, 1)):
            a = A[:, i * 3 + j].astype(np.float64)
            for q in range(4):
                C12[:, q] += a
            if sh == -1:
                C12[:, 5] -= a
            elif sh == 1:
                C12[:, 4] -= a
            if sw == -1:
                C12[:, 7] -= a
            elif sw == 1:
                C12[:, 6] -= a
            if sh != 0 and sw != 0:
                rr = 95 if sh == -1 else 0
                cc = 95 if sw == -1 else 0
                idx = 8 + (2 if rr == 95 else 0) + (1 if cc == 95 else 0)
                C12[:, idx] += a
    C12 = C12.astype(np.float32)

    pp = np.zeros((NCB, 128, 14), np.float32)
    ppb = np.zeros((NCB, 128, 8), bf16np)
    for cb in range(NCB):
        s = slice(cb * 128, (cb + 1) * 128)
        pp[cb, :, 0:12] = C12[s]
        pp[cb, :, 12] = d[s]
        pp[cb, :, 13] = shift[s]
        ppb[cb] = cls_dense_w[:, s].T.astype(bf16np)

    ke = kernel_embed.reshape(E, C, 9) * inv[None, :, None]
    be = bias_embed * inv[None, :]
    emb = np.zeros((E, NCB * 10 * 128), bf16np)
    for cb in range(NCB):
        s = slice(cb * 128, (cb + 1) * 128)
        for t in range(9):
            emb[:, (cb * 10 + t) * 128:(cb * 10 + t + 1) * 128] = \
                ke[:, s, t].astype(bf16np)
        emb[:, (cb * 10 + 9) * 128:(cb * 10 + 10) * 128] = \
            be[:, s].astype(bf16np)

    db = cls_dense_b.reshape(1, E).astype(np.float32)
    eye = np.eye(128, dtype=np.float32) * 16.0
    id8 = np.concatenate([eye, eye], axis=1).astype(f8np)

    blob = np.zeros((128, BLOB_B), np.uint8)
    for cb in range(NCB):
        blob[:, OFF_PP[cb]:OFF_PP[cb] + 56] = \
            pp[cb].view(np.uint8).reshape(128, 56)
        blob[:, OFF_PPB[cb]:OFF_PPB[cb] + 16] = \
            ppb[cb].view(np.uint8).reshape(128, 16)
    blob[0, OFF_DB:OFF_DB + 32] = db.view(np.uint8).reshape(32)
    blob[:, OFF_ID8:OFF_ID8 + 256] = id8.view(np.uint8)
    blob[0:8, OFF_EMB:BLOB_B] = emb.view(np.uint8).reshape(8, -1)
    return blob


def kernel(x, kernel_embed, bias_embed, cls_conv_w, cls_conv_b,
           cls_bn_gamma, cls_bn_beta, cls_bn_mean, cls_bn_var,
           cls_dense_w, cls_dense_b, bn_gamma, bn_beta, bn_mean, bn_var,
           _trace=False, _trace_kwargs=None):
    from concourse.mybir import dt as _dt
    f8np = _dt.np(f8)

    x = np.asarray(x, dtype=np.float32)
    xp_full = np.zeros((B, C, H, WP), np.float32)
    xp_full[:, :, :, 1:97] = x
    x8_full = xp_full.astype(f8np)
    r8_full = (xp_full - x8_full.astype(np.float32)).astype(f8np)
    args = [np.asarray(a, dtype=np.float32) for a in
            (kernel_embed, bias_embed, cls_conv_w, cls_conv_b,
             cls_bn_gamma, cls_bn_beta, cls_bn_mean, cls_bn_var,
             cls_dense_w, cls_dense_b, bn_gamma, bn_beta, bn_mean, bn_var)]
    blob = _host_prep(*args)

    nc = _get_nc()
    in_maps = []
    for core in range(NCORES):
        s = slice(core * BLOC, (core + 1) * BLOC)
        in_maps.append({"x8": np.ascontiguousarray(x8_full[s]),
                        "r8": np.ascontiguousarray(r8_full[s]),
                        "blob": blob})
    kw = {}
    if _trace:
        kw["trace"] = True
        if _trace_kwargs:
            kw.update(_trace_kwargs)
    res = run_bass_kernel_spmd(nc, in_maps, core_ids=list(range(NCORES)), **kw)
    out = np.concatenate(
        [res.results[i]["out"].astype(np.float32) for i in range(NCORES)],
        axis=0)
    if _trace:
        return out, res
    return out
